# revision 2
# baseline (speedup 1.0000x reference)
"""Luong 'general' attention scores + softmax on 8 Trainium2 NeuronCores.

Reference computes:
    energy = einsum('sbh,kh->sbk', enc, W) + b          # [S,B,H]
    scores = einsum('bh,sbh->bs', hidden[0], energy)    # [B,S]
    attn   = softmax(scores, axis=1)[:, None, :]        # [B,1,S]

Algebra: scores[b,s] = hidden[b] . (W @ enc[s,b]) + hidden[b] . bias.
The bias term is constant over s, so it cancels in the softmax.  With
q = hidden @ W  (tiny [B,H]x[H,H] matmul), scores[b,s] = q[b] . enc[s,b].
The kernel is therefore HBM-bound on streaming enc (268 MB) exactly once.

Sharding: data-parallel over batch.  Core c gets batches [16c, 16c+16).
SBUF partitions pack (group g in [0,8)) x (batch b in [0,16)); group g owns
s in [64g, 64g+64).  Inputs are host-permuted to partition-major layouts so
every DMA is a dense 128-partition transfer:
    enc_dev[g*16+b, c*H+h] = enc[g*64+c, 16*core+b, h]
    w_dev[p, kc*H+h]       = W[kc*128+p, h]
    hidT[p, kc*16+b]       = hidden[0, 16*core+b, kc*128+p]
Per s-column, a DVE tensor_tensor multiply against broadcast q plus a
ScalarE activation(Copy, accum_out) free-dim sum produce all 128 scores.
enc streams through the SP HWDGE ring (even chunks) and SWDGE (odd chunks),
leaving the ACT ring for W and the accumulating reduces.  Softmax runs on a
[16, 512] rearranged tile (exp+sum fused in one ScalarE activation).
"""

import os
import sys

for _p in ("/opt/trn_rl_repo", "/root/.axon_site/_ro/trn_rl_repo"):
    if os.path.isdir(_p):
        sys.path.insert(0, _p)
        break

from contextlib import ExitStack

import numpy as np

import concourse.tile as tile
from concourse import bacc, mybir
from concourse.bass_utils import run_bass_kernel_spmd

S, B, H = 512, 128, 1024
NCORES = 8
BLOC = B // NCORES          # 16 batches per core
GROUPS = 8                  # partition groups; GROUPS * BLOC = 128
S_PER_GROUP = S // GROUPS   # 64 s-values per group
KC = H // 128               # 8 contraction chunks for q = hidden @ W
CHUNK = 8                   # s-columns per enc DMA (4 MB per transfer)
NCHUNK = S_PER_GROUP // CHUNK

FP32 = mybir.dt.float32

_cache = {}
LAST_RESULTS = None  # test harness reads exec_time_ns off this


def _build_nc():
    if "nc" in _cache:
        return _cache["nc"]

    # Bacc (not raw Bass): its compile pipeline legalizes sync waits to the
    # TRN2 1-wait-per-instruction limit and encodes InstISA subclasses.
    nc = bacc.Bacc(
        "TRN2",
        target_bir_lowering=False,
        debug=False,
        enable_asserts=True,
        num_devices=NCORES,
    )
    enc_d = nc.dram_tensor(
        "enc", [128, S_PER_GROUP * H], FP32, kind="ExternalInput"
    ).ap()
    # W in two h-halves so each HWDGE ring loads one in parallel:
    # w{0,1}[p, kc*512 + j] = W[kc*128 + p, half*512 + j].
    # hidT_rep (hidT_rep[p, kc*128 + g*16 + b] = hidden[b, kc*128+p], i.e.
    # hidT with columns tiled 8x over the groups) is packed into the HEAD of
    # w0: with a 128-wide lhsT the q matmuls produce the group-broadcast
    # qb[128, 512] directly in PSUM — no separate partition broadcast.
    w0_d = nc.dram_tensor(
        "w0", [128, KC * 128 + KC * 512], FP32, kind="ExternalInput"
    ).ap()
    w1_d = nc.dram_tensor("w1", [128, KC * 512], FP32, kind="ExternalInput").ap()
    out = nc.dram_tensor("attn", [BLOC, S], FP32, kind="ExternalOutput").ap()

    with tile.TileContext(nc) as tc, ExitStack() as ctx:
        const_pool = ctx.enter_context(tc.tile_pool(name="const", bufs=1))
        w_pool = ctx.enter_context(tc.tile_pool(name="w", bufs=1))
        enc_pool = ctx.enter_context(tc.tile_pool(name="enc", bufs=4))
        scratch_pool = ctx.enter_context(tc.tile_pool(name="scratch", bufs=3))
        small_pool = ctx.enter_context(tc.tile_pool(name="small", bufs=1))
        psum_pool = ctx.enter_context(tc.tile_pool(name="psum", bufs=2, space="PSUM"))

        # ---- Phase 0: qb = broadcast(hidden @ W) straight out of PE ----
        # PE clock-gate warmup: dummy matmuls on a memset tile during the
        # W-load window so the real q matmuls run at the warm 2.4 GHz clock.
        wu = const_pool.tile([128, 512], FP32)
        nc.gpsimd.memset(wu[:], 1.0)
        wp = psum_pool.tile([1, 512], FP32, tag="wu")
        for _ in range(2):
            nc.tensor.matmul(wp[:], wu[:, 0:1], wu[:], start=True, stop=True)

        # Staged loads: the first piece is just hidT_rep + kc0, so the PE
        # pipeline starts after ~2.5 us instead of waiting for all of W.
        w0_sb = w_pool.tile([128, KC * 128 + KC * 512], FP32)
        w1_sb = w_pool.tile([128, KC * 512], FP32)
        cut_a = KC * 128 + 512          # hidT_rep + kc0 of half 0
        cut_b = KC * 128 + 4 * 512      # kc1..3 of half 0
        nc.sync.dma_start(w0_sb[:, :cut_a], w0_d[:, :cut_a])
        nc.sync.dma_start(w0_sb[:, cut_a:cut_b], w0_d[:, cut_a:cut_b])
        nc.sync.dma_start(w0_sb[:, cut_b:], w0_d[:, cut_b:])
        mid1 = (KC // 2) * 512
        nc.scalar.dma_start(w1_sb[:, :mid1], w1_d[:, :mid1])
        nc.scalar.dma_start(w1_sb[:, mid1:], w1_d[:, mid1:])
        hidT = w0_sb[:, : KC * 128]
        w_half = [w0_sb[:, KC * 128 :], w1_sb[:]]

        qb = const_pool.tile([128, H], FP32)
        for half in range(2):
            qp = psum_pool.tile([128, 512], FP32, tag="qp")
            for kc in range(KC):
                nc.tensor.matmul(
                    qp[:],
                    hidT[:, kc * 128 : (kc + 1) * 128],
                    w_half[half][:, kc * 512 : (kc + 1) * 512],
                    start=(kc == 0),
                    stop=(kc == KC - 1),
                )
            nc.scalar.copy(qb[:, half * 512 : (half + 1) * 512], qp[:])

        # ---- Phase 1: stream enc, multiply (DVE) + accum-reduce (ACT) ----
        # scores[g*16+b, c] = q[b] . enc[g*64+c, b].
        # (tensor_tensor_reduce would fuse both in one DVE op, but it
        # crashes the device runtime in this toolchain; TT + activation
        # accum splits the work across DVE and the otherwise-idle ScalarE
        # at the same DVE cost.)  enc chunks: even ones on the SP HWDGE
        # ring, odd ones via SWDGE, keeping the ACT ring free for the 64
        # accumulating reduces.
        scores = small_pool.tile([128, S_PER_GROUP], FP32)
        for ch in range(NCHUNK):
            et = enc_pool.tile([128, CHUNK * H], FP32, tag="enc")
            eng = nc.gpsimd if ch % 2 == 0 else nc.sync
            eng.dma_start(et[:], enc_d[:, ch * CHUNK * H : (ch + 1) * CHUNK * H])
            for j in range(CHUNK):
                col = ch * CHUNK + j
                prod = scratch_pool.tile([128, H], FP32, tag="prod")
                nc.vector.tensor_tensor(
                    out=prod[:],
                    in0=et[:, j * H : (j + 1) * H],
                    in1=qb[:],
                    op=mybir.AluOpType.mult,
                )
                # Dead output written through a step-0 broadcast AP — only
                # accum_out matters; saves 12 KB/partition of SBUF.
                ascr = scratch_pool.tile([128, 1], FP32, tag="ascr")
                nc.scalar.activation(
                    ascr[:].broadcast_to([128, H]),
                    prod[:],
                    mybir.ActivationFunctionType.Copy,
                    accum_out=scores[:, col : col + 1],
                )

        # ---- Phase 2: softmax over s per batch ----
        # SP HWDGE ring is idle by now; it carries the rearrange + output.
        scoresT = small_pool.tile([BLOC, S], FP32)
        for g in range(GROUPS):
            eng = nc.sync if g % 2 == 0 else nc.scalar
            eng.dma_start(
                scoresT[:, g * S_PER_GROUP : (g + 1) * S_PER_GROUP],
                scores[g * BLOC : (g + 1) * BLOC, :],
            )
        mx = small_pool.tile([BLOC, 1], FP32)
        nc.vector.reduce_max(mx[:], scoresT[:], axis=mybir.AxisListType.X)
        nmx = small_pool.tile([BLOC, 1], FP32)
        nc.vector.tensor_scalar_mul(nmx[:], mx[:], -1.0)
        probs = small_pool.tile([BLOC, S], FP32)
        ssum = small_pool.tile([BLOC, 1], FP32)
        nc.scalar.activation(
            probs[:],
            scoresT[:],
            mybir.ActivationFunctionType.Exp,
            bias=nmx[:],
            scale=1.0,
            accum_out=ssum[:],
        )
        rsum = small_pool.tile([BLOC, 1], FP32)
        nc.vector.reciprocal(rsum[:], ssum[:])
        attn_sb = small_pool.tile([BLOC, S], FP32)
        nc.scalar.mul(attn_sb[:], probs[:], rsum[:])
        nc.sync.dma_start(out, attn_sb[:])

    nc.finalize()
    _cache["nc"] = nc
    return nc


def _prep_core_inputs(hidden, enc, w_dev, c):
    b0 = c * BLOC
    hl = hidden[0, b0 : b0 + BLOC, :]  # [16, 1024]
    hidT = hl.reshape(BLOC, KC, 128).transpose(2, 1, 0)  # [128, KC, 16]
    hidT_rep = np.tile(hidT, (1, 1, GROUPS)).reshape(128, KC * 128)
    el = enc[:, b0 : b0 + BLOC, :]  # [512, 16, 1024]
    encd = np.ascontiguousarray(
        el.reshape(GROUPS, S_PER_GROUP, BLOC, H)
        .transpose(0, 2, 1, 3)
        .reshape(128, S_PER_GROUP * H)
    )
    w0 = np.ascontiguousarray(np.concatenate([hidT_rep, w_dev[0]], axis=1))
    return {"enc": encd, "w0": w0, "w1": w_dev[1]}


def _warmup():
    """Compile + run once on dummy inputs at import time so the first real
    kernel() call hits the in-process XLA/NEFF caches instead of paying the
    multi-minute compile."""
    if _cache.get("warm"):
        return
    try:
        kernel(
            np.zeros((1, B, H), np.float32),
            np.zeros((S, B, H), np.float32),
            np.zeros((H, H), np.float32),
            np.zeros((H,), np.float32),
        )
        _cache["warm"] = True
    except Exception:
        pass


def _prep_in_maps(inputs):
    hidden = np.asarray(inputs["hidden"], dtype=np.float32)
    enc = np.asarray(inputs["encoder_outputs"], dtype=np.float32)
    w = np.asarray(inputs["W_attn"], dtype=np.float32)
    wr = w.reshape(KC, 128, H).transpose(1, 0, 2)  # [128, KC, H]
    w_dev = (
        np.ascontiguousarray(wr[:, :, :512].reshape(128, KC * 512)),
        np.ascontiguousarray(wr[:, :, 512:].reshape(128, KC * 512)),
    )
    return [_prep_core_inputs(hidden, enc, w_dev, c) for c in range(NCORES)]


def kernel(hidden, encoder_outputs, W_attn, b_attn=None, **_unused):
    global LAST_RESULTS
    nc = _build_nc()
    in_maps = _prep_in_maps(
        {"hidden": hidden, "encoder_outputs": encoder_outputs, "W_attn": W_attn}
    )
    res = run_bass_kernel_spmd(nc, in_maps, core_ids=list(range(NCORES)))
    LAST_RESULTS = res
    attn = np.concatenate([res.results[c]["attn"] for c in range(NCORES)], axis=0)
    return attn[:, None, :].astype(np.float32)


_warmup()

